# revision 97
# baseline (speedup 1.0000x reference)
"""Trainium2 Bass kernel for nn_CrossAttention (MLA-style cross attention).

Sharding: 8 cores = 2 batches x 4 head-groups (4 heads each).

v2 design:
- Host fuses the down+up projections (w_dq@w_uq etc.) so every projection is
  a single Z=1024-contraction matmul, head-sharded across cores (no
  replicated down-projection work).
- All device activations feature-major [dims, seq]; scores computed
  transposed ([k, q]) so softmax sums come from an appended ones-column in V.
- PV is computed output-transposed: psum [q, 65] accumulated over k-chunks
  (65 rows/instr instead of 512), normalized per-partition on DVE, then
  PE-transposed (identity matmul) back to [d, q] for the fc.
- RoPE partner term via DMA partition pair-swap (sign baked into host
  tables) instead of a second matmul.
- Act engine runs only the exp activations; psum drains go to Pool/DVE.
- Emission interleaves projection/fc matmuls into the attention k-loop so
  the PE stays busy while Act works through the exps.
"""

import math
from contextlib import ExitStack

import numpy as np
import ml_dtypes

import concourse.bass as bass
import concourse.tile as tile
from concourse import bacc, mybir
from concourse.bass_utils import run_bass_kernel_spmd

bf16 = ml_dtypes.bfloat16
F32 = mybir.dt.float32
BF = mybir.dt.bfloat16

# problem constants (hardcoded per contract)
B, S, Z, DOWN, UP, H, RHD, VHD = 2, 2048, 1024, 512, 1024, 16, 64, 64
HPC = 4            # heads per core
NCORES = 8
SCALE = 1.0 / (math.sqrt(64) + math.sqrt(64))  # 1/16

_cache = {}


def _rope_tables():
    theta = 1.0 / (10000.0 ** (np.arange(0, RHD, 2, dtype=np.float32) / RHD))
    pe = np.arange(S, dtype=np.float32)[:, None] * theta[None, :]  # [S, 32]
    # faithful to reference: cos_pos stores sin, sin_pos stores cos
    CT = np.repeat(np.sin(pe), 2, axis=-1).T.astype(np.float32)  # [64, S]
    ST = np.repeat(np.cos(pe), 2, axis=-1).T.astype(np.float32)
    return CT, ST


def _partner_rows(v):
    """p[2i] = -v[2i+1]; p[2i+1] = v[2i] (on first axis)."""
    p = np.empty_like(v)
    p[0::2] = -v[1::2]
    p[1::2] = v[0::2]
    return p


def build_nc(with_bias):
    nc = bacc.Bacc("TRN2", target_bir_lowering=False, debug=False,
                   num_devices=NCORES)

    def din(name, shape, dt=BF):
        return nc.dram_tensor(name, shape, dt, kind="ExternalInput").ap()

    qT = din("qT", [Z, S])
    kT = din("kT", [Z, S])
    wq = din("wq", [Z, 512])      # per head [fused uq 64 | fused qr 64]
    wk = din("wk", [Z, 256])      # per head-pair [uk(h even) | uk(h odd)]
    wv = din("wv", [Z, 256])      # head-major fused uv cols
    wkr = din("wkr", [Z, 64])
    # rope tables, duplicated into both partition halves so ops on rows
    # 64:128 see base-aligned operands (BIR: SBUF operands of one op must
    # share their start partition)
    ct1 = din("ct1", [128, S])    # repeat(sin(pe),2) rows, stacked twice
    st1 = din("st1", [128, S])    # +-repeat(cos(pe),2) rows, stacked twice
    wfc = din("wfc", [256, Z])
    ident = din("ident", [128, 128])
    if with_bias:
        biasq = din("biasq", [128, HPC, S])
        biask4 = din("biask4", [128, HPC, S])
    outT = nc.dram_tensor("outT", [Z, S], BF, kind="ExternalOutput").ap()

    NSC = 4            # 512-wide S chunks
    CW = 512

    with tile.TileContext(nc) as tc, ExitStack() as ctx:
        # ---------------- static pools ----------------
        sp = ctx.enter_context(tc.tile_pool(name="static", bufs=1))

        def stile(shape, dt, name):
            return sp.tile(shape, dt, name=name, tag=name)

        wq_sb = stile([128, 8, 512], BF, "wq_sb")
        wk_sb = stile([128, 8, 256], BF, "wk_sb")
        wv_sb = stile([128, 8, 256], BF, "wv_sb")
        wkr_sb = stile([128, 8, 64], BF, "wkr_sb")
        wfc_sb = stile([128, 2, 8, 128], BF, "wfc_sb")
        ct_sb = stile([128, S], BF, "ct_sb")
        st_sb = stile([128, S], BF, "st_sb")
        id_sb = stile([128, 128], BF, "id_sb")

        qT_sb = stile([128, 8, S], BF, "qT_sb")
        kT_sb = stile([128, 8, S], BF, "kT_sb")

        qcat_sb = stile([128, 4, S], BF, "qcat_sb")
        kcat_sb = stile([128, 4, S], BF, "kcat_sb")
        va_sb = stile([128, 16, HPC * 65], BF, "va_sb")
        va_v = va_sb.rearrange("p sc (h e) -> p sc h e", e=65)
        af_sb = stile([128, 2, S], BF, "af_sb")

        kr_raw = stile([128, S], BF, "kr_raw")    # rows 64:128 used
        kr_swap = stile([128, S], BF, "kr_swap")

        # DMA order: what the prologue consumes first goes first.
        qT_r = qT.rearrange("(c p) s -> p c s", p=128)
        kT_r = kT.rearrange("(c p) s -> p c s", p=128)

        def in_chunk(sc):
            # first chunks go down the Act HWDGE queue so they don't sit
            # behind the weight DMAs on the SP queue; later chunks ride SP.
            # chunk 0 is split by zc halves so the first projection's
            # accumulation can start after half the transfer.
            eng = nc.scalar if sc < 2 else nc.sync
            ssl = slice(CW * sc, CW * (sc + 1))
            eng.dma_start(kT_sb[:, :, ssl], kT_r[:, :, ssl])
            eng.dma_start(qT_sb[:, :, ssl], qT_r[:, :, ssl])

        for w_sb, w in ((wk_sb, wk), (wv_sb, wv), (wkr_sb, wkr)):
            nc.sync.dma_start(w_sb[:], w.rearrange("(c p) m -> p c m", p=128))
        nc.sync.dma_start(ct_sb[:], ct1[:])
        nc.sync.dma_start(st_sb[:], st1[:])
        in_chunk(0)
        nc.sync.dma_start(id_sb[:], ident[:])
        nc.sync.dma_start(wq_sb[:], wq.rearrange("(c p) m -> p c m", p=128))
        # wfc is not needed until the first fc window; emitted late so body
        # DMAs (rope swaps) aren't stuck behind it on the SP queue.
        wfc_dma = [lambda: nc.sync.dma_start(wfc_sb[:], wfc.rearrange(
            "(c p) (z m) -> p c z m", p=128, m=128))]
        if with_bias:
            bq_pool = ctx.enter_context(tc.tile_pool(name="bq", bufs=1))
            biasq_sb = bq_pool.tile([128, HPC, S], BF, name="biasq_sb",
                                    tag="biasq_sb")
            biask_sb = bq_pool.tile([128, HPC, S], BF, name="biask_sb",
                                    tag="biask_sb")
            nc.sync.dma_start(biasq_sb[:], biasq[:])
            nc.sync.dma_start(biask_sb[:], biask4[:])
        in_chunk(1)

        # ---------------- psum pools ----------------
        # scores+pv pools live only for the attention windows (scoped below)
        # so the tail fc can reuse their banks.
        fx_pool = ctx.enter_context(tc.tile_pool(name="fxp", bufs=2,
                                                 space="PSUM"))
        wrk = ctx.enter_context(tc.tile_pool(name="wrk", bufs=3))
        pools = {}

        def ps_scores():
            return pools["sc"].tile([128, 1024], F32, name="ps_sc",
                                    tag="ps_sc")

        def ps_pv():
            t = pools["pv"].tile([128, 1024], F32, name="ps_pv", tag="ps_pv")
            return t.rearrange("p (qc e) -> p qc e", e=128)

        def ps_flex():
            return fx_pool.tile([128, 512], F32, name="ps_fx", tag="ps_fx")

        def ps_tr():
            # transpose staging reuses a flex-ring bank, viewed as bf16
            return ps_flex()[0:64, :].bitcast(BF)

        # psum->sbuf drains: GPSIMD cannot touch PSUM, so these live on DVE
        # (rope combines go to Pool instead); the tail alternates DVE/Act
        # since the exps are done by then.
        state = {"att_started": False, "alt": 0, "tail": False}

        def drain_copy(dst, src):
            state["alt"] ^= 1
            if state["tail"] and state["alt"]:
                nc.scalar.copy(dst, src)
            else:
                nc.vector.tensor_copy(dst, src)

        # ---------------- side-work generators ----------------
        def rope_combine(dst, raw, swp, csl):
            """dst = raw*ct + swap*st over S-slice csl. All operands sit on
            partitions 64:128 (BIR wants SBUF operands base-aligned).
            All-SBUF bf16: DVE (2x, 3.4x faster per op) while the prologue
            chain is latency-critical, Pool once attention is running."""
            eng = nc.gpsimd
            tt = wrk.tile([128, CW], BF, name="ropet", tag="ropet", bufs=3)
            eng.tensor_tensor(dst, raw, ct_sb[64:128, csl],
                              mybir.AluOpType.mult)
            eng.tensor_tensor(tt[64:128, :], swp, st_sb[64:128, csl],
                              mybir.AluOpType.mult)
            eng.tensor_tensor(dst, dst, tt[64:128, :],
                              mybir.AluOpType.add)

        SWAP_MASK = [i ^ 1 for i in range(32)]

        def swap_dma(dst, src):
            """partition pair-swap: dst[2i]=src[2i+1], dst[2i+1]=src[2i].
            stream_shuffle permutes partitions within 32-blocks; the
            pair-swap mask is an involution so direction is irrelevant."""
            nc.vector.stream_shuffle(dst, src, SWAP_MASK)

        ks_prog = {"sc": 0}

        def gen_kside():
            """k_c, v_aug and k-rope per S-chunk. yields ~1024-row credit
            units so the pump can spread work finely."""
            nc.vector.memset(va_sb[:, :, 64::65], 1.0)
            for sc in range(NSC):
                ssl = slice(CW * sc, CW * (sc + 1))
                # k_c: head pairs packed on psum partitions
                for pair in range(2):
                    ps = ps_flex()
                    for zc in range(8):
                        nc.tensor.matmul(
                            ps[:], wk_sb[:, zc, 128 * pair:128 * (pair + 1)],
                            kT_sb[:, zc, ssl], start=(zc == 0), stop=(zc == 7))
                        if zc % 2 == 1:
                            yield 1024
                    for sub in range(2):
                        h = 2 * pair + sub
                        kd = kcat_sb[0:64, h, ssl]
                        drain_copy(kd, ps[64 * sub:64 * (sub + 1), :])
                        if with_bias:
                            nc.vector.tensor_tensor(
                                kd, kd, biask_sb[0:64, h, ssl],
                                mybir.AluOpType.add)
                    yield 256
                # v_aug: psum [s, v] via flipped matmul, 128-s at a time
                for sub in range(4):
                    sck = 4 * sc + sub
                    ps = ps_flex()
                    for zc in range(8):
                        nc.tensor.matmul(
                            ps[:, 0:256],
                            kT_sb[:, zc, 128 * sck:128 * (sck + 1)],
                            wv_sb[:, zc, :],
                            start=(zc == 0), stop=(zc == 7))
                        if zc % 4 == 3:
                            yield 1024
                    drain_copy(
                        va_v[:, sck, :, 0:64],
                        ps[:, 0:256].rearrange("p (h e) -> p h e", e=64))
                    yield 256
                # k-rope raw -> sbuf, pair-swap, combine into kcat head 0
                ps = ps_flex()
                for zc in range(8):
                    nc.tensor.matmul(ps[0:64, :], wkr_sb[:, zc, :],
                                     kT_sb[:, zc, ssl],
                                     start=(zc == 0), stop=(zc == 7))
                    if zc % 2 == 1:
                        yield 1024
                drain_copy(kr_raw[64:128, ssl], ps[0:64, :])
                swap_dma(kr_swap[64:128, ssl], kr_raw[64:128, ssl])
                rope_combine(kcat_sb[64:128, 0, ssl], kr_raw[64:128, ssl],
                             kr_swap[64:128, ssl], ssl)
                if with_bias:
                    nc.vector.tensor_tensor(
                        kcat_sb[64:128, 0, ssl], kcat_sb[64:128, 0, ssl],
                        biask_sb[64:128, 0, ssl], mybir.AluOpType.add)
                # duplicate rope rows (incl. bias: same for all heads)
                for h in range(1, HPC):
                    nc.sync.dma_start(kcat_sb[64:128, h, ssl],
                                      kcat_sb[64:128, 0, ssl])
                ks_prog["sc"] = sc + 1
                yield 256

        def gen_qside(h, qh):
            """qcat for head h, S-half qh (1024 wide = 2 chunks)."""
            for c in range(2):
                sc = 2 * qh + c
                ssl = slice(CW * sc, CW * (sc + 1))
                ps = ps_flex()
                for zc in range(8):
                    nc.tensor.matmul(
                        ps[:], wq_sb[:, zc, 128 * h:128 * (h + 1)],
                        qT_sb[:, zc, ssl], start=(zc == 0), stop=(zc == 7))
                yield 4096
                qd = qcat_sb[:, h, ssl]
                drain_copy(qd[0:64, :], ps[0:64, :])
                raw = wrk.tile([128, CW], BF, name="qr_raw", tag="qr_raw",
                               bufs=2)
                swp = wrk.tile([128, CW], BF, name="qr_swap", tag="qr_swap",
                               bufs=2)
                drain_copy(raw[64:128, :], ps[64:128, :])
                swap_dma(swp[64:128, :], raw[64:128, :])
                rope_combine(qd[64:128, :], raw[64:128, :], swp[64:128, :],
                             ssl)
                if with_bias:
                    nc.vector.tensor_tensor(qd, qd, biasq_sb[:, h, ssl],
                                            mybir.AluOpType.add)
                yield 0

        def gen_fc(qc4, psfn=None, alt_drain=False):
            """fc for one 512-wide q chunk; af[:, :, chunk] must be done."""
            qsl = slice(CW * qc4, CW * (qc4 + 1))
            for zc in range(8):
                ps = (psfn or ps_flex)()
                for c in range(2):
                    nc.tensor.matmul(ps[:], wfc_sb[:, c, zc, :],
                                     af_sb[:, c, qsl],
                                     start=(c == 0), stop=(c == 1))
                yield 1024
                ob = wrk.tile([128, 512], BF, name="ob", tag="ob", bufs=4)
                if alt_drain and zc % 2 == 0:
                    nc.vector.tensor_copy(ob[:], ps[:])
                else:
                    drain_copy(ob[:], ps[:])
                nc.sync.dma_start(outT[128 * zc:128 * (zc + 1), qsl], ob[:])
                yield 0

        # ---------------- attention window ----------------
        kside = gen_kside()
        ks_done = {"v": False}

        def pump_kside(upto_sc, steps=0):
            n = 0
            while not ks_done["v"]:
                if ks_prog["sc"] >= upto_sc and n >= steps:
                    return
                try:
                    next(kside)
                except StopIteration:
                    ks_done["v"] = True
                n += 1

        def finish_window(prev, deferred):
            """normalize prev window's PV psum (frees it) and defer its
            transpose epilogue. Called from the NEXT window's top, right
            after its scores(0), so Act's first exp is never delayed."""
            ph, pqb, pqcs, ppv = prev
            rec = wrk.tile([128, 8, 1], F32, name="rec", tag="rec", bufs=2)
            nc.vector.reciprocal_approx_fast(rec[:, 0:pqcs, :],
                                             ppv[:, 0:pqcs, 64:65])
            att = wrk.tile([128, 8, 64], BF, name="att", tag="att", bufs=2)
            nc.vector.tensor_tensor(
                att[:, 0:pqcs, :], ppv[:, 0:pqcs, 0:64],
                rec[:, 0:pqcs, :].to_broadcast([128, pqcs, 64]),
                mybir.AluOpType.mult)

            def epilogue():
                tp = ps_tr()
                for qc in range(pqcs):
                    nc.tensor.matmul(tp[:, 128 * qc:128 * (qc + 1)],
                                     att[:, qc, :], id_sb[:],
                                     is_transpose=True,
                                     start=(qc == 0), stop=(qc == pqcs - 1))
                ro = slice(0, 64) if ph % 2 == 0 else slice(64, 128)
                nc.vector.tensor_copy(
                    af_sb[ro, ph // 2, pqb:pqb + 128 * pqcs],
                    tp[:, 0:128 * pqcs])

            deferred.append(epilogue)

        def attention(h, qb, side, credit_per_kc, deferred, prev, qcs=8,
                      epi_now=False):
            """scores+exp+PV for head h over q range [qb, qb+128*qcs),
            software-pipelined: per kc emit scores(kc)/exp(kc) before
            PV(kc-2) so the in-order PE never parks on the exp PV needs."""
            qw = 128 * qcs

            def scores(kc):
                sc_ps = ps_scores()
                for half in range(qw // 512):
                    psl = slice(512 * half, 512 * (half + 1))
                    rsl = slice(qb + 512 * half, qb + 512 * (half + 1))
                    nc.tensor.matmul(
                        sc_ps[:, psl],
                        kcat_sb[:, h, 128 * kc:128 * (kc + 1)],
                        qcat_sb[:, h, rsl], start=True, stop=True)
                pr = wrk.tile([128, 1024], BF, name="pr", tag="pr", bufs=6)
                state["att_started"] = True
                nc.scalar.activation(pr[:, 0:qw], sc_ps[:, 0:qw],
                                     mybir.ActivationFunctionType.Exp,
                                     scale=SCALE)
                return pr

            def pvmm(kc, pr):
                # psum accumulation groups are 2KB-zero-region granular:
                # one start/stop per bank (qc 0-3 share a bank, 4-7 the
                # other); the start's pending-zero makes qc 1-3 overwrite.
                for qc in range(qcs):
                    nc.tensor.matmul(
                        pv[:, qc, 0:65],
                        pr[:, 128 * qc:128 * (qc + 1)],
                        va_v[:, kc, h, :],
                        start=(kc == 0 and qc % 4 == 0),
                        stop=(kc == 15 and qc % 4 == min(3, qcs - 1)))

            prs = [scores(0)]
            # previous window's normalize is emitted only now (after our
            # scores(0)/exp(0)), then the PV psum can be reused
            if prev is not None:
                finish_window(prev, deferred)
            pv = ps_pv()
            if epi_now:
                # run the epilogue immediately (its af output feeds side
                # work already queued for this window, e.g. the fc of the
                # q-chunk the previous half-window just finished)
                while deferred:
                    deferred.pop(0)()
            pull = 3600
            while pull > 0 and side:
                try:
                    pull -= max(next(side[0]), 256)
                except StopIteration:
                    side.pop(0)
            # 3-deep pipeline: PV(kc-2) is emitted after scores(kc+1); the
            # previous window's transpose epilogue runs at kc==2, when its
            # normalize has surely drained.
            for kc in range(16):
                if kc < 15:
                    # kside chunk (kc+1)//4 must be emitted before the
                    # scores that read it
                    if not ks_done["v"]:
                        pump_kside((kc + 1) // 4 + 1, steps=6)
                    prs.append(scores(kc + 1))
                if kc == 2:
                    while deferred:
                        deferred.pop(0)()
                if kc >= 2:
                    pvmm(kc - 2, prs[kc - 2])
                pull = credit_per_kc
                while pull > 0 and side:
                    try:
                        pull -= max(next(side[0]), 256)
                    except StopIteration:
                        side.pop(0)
            pvmm(14, prs[14])
            pvmm(15, prs[15])
            return (h, qb, qcs, pv)

        # ---------------- emission schedule ----------------
        qsides = {(h, qh): gen_qside(h, qh) for h in range(HPC)
                  for qh in range(2)}
        fcs = {c: gen_fc(c) for c in range(NSC)}

        def flush(g):
            for _ in g:
                pass

        side = []            # list of generators, pulled front-first
        queued = set()       # qside keys already handed to side
        deferred = []        # PE epilogue closures from previous window

        def finish_qside(key):
            g = qsides.pop(key, None)
            if g is None:
                return
            if g in side:
                side.remove(g)
            flush(g)

        with tc.tile_pool(name="scp", bufs=2, space="PSUM") as scp, \
             tc.tile_pool(name="pvp", bufs=1, space="PSUM") as pvp:
            pools["sc"], pools["pv"] = scp, pvp

            # prologue: k-side chunk 0 plus head-0 qh0 projections.
            pump_kside(1)
            finish_qside((0, 0))
            # later input chunks ride the SP queue after the sc0 rope swaps
            in_chunk(2)
            in_chunk(3)

            # windows run qh-outer so fc(q-half 0) can interleave into the
            # qh=1 windows; side queue feeds PE during Act-bound stretches.
            # The final (3,1) window is split into two 512-wide halves so
            # fc(chunk 2) overlaps the second half instead of the tail.
            wins = [(h, 1024 * qh, 8) for qh in range(2) for h in range(HPC)]
            wins = wins[:-1] + [(3, 1024, 4), (3, 1536, 4)]
            prev = None
            for i, (h, qb, qcs) in enumerate(wins):
                # qcat consumed by this window must be emitted already
                finish_qside((h, qb // 1024))
                # enqueue upcoming side work (next window's qcat; fc for
                # the completed q-half once windows reach qh=1)
                if i + 1 < len(wins):
                    nh, nqb, _ = wins[i + 1]
                    nxt = (nh, nqb // 1024)
                    if nxt in qsides and nxt not in queued:
                        side.append(qsides[nxt])
                        queued.add(nxt)
                if i == 2 and wfc_dma:
                    wfc_dma.pop()()
                epi_now = False
                if qb >= 1024:
                    # fc chunk 0 into window (1,1); chunk 1 into the first
                    # (3,1) half; chunk 2 into the last half-window right
                    # after its af chunk completes (epilogue runs early)
                    c = None
                    if (h, qcs) == (1, 8):
                        c = 0
                    elif (h, qcs) == (3, 4):
                        c = 1 if qb == 1024 else 2
                    if c == 2 and c in fcs:
                        side.insert(0, fcs.pop(c))
                        epi_now = True
                    elif c is not None and c in fcs:
                        side.append(fcs.pop(c))
                prev = attention(h, qb, side, 1100, deferred, prev,
                                 qcs=qcs, epi_now=epi_now)

            # drain stragglers that must use the windowed pools
            finish_window(prev, deferred)
            pump_kside(NSC)
            while deferred:
                deferred.pop(0)()
            for key in list(qsides):
                finish_qside(key)
            for g in list(side):
                side.remove(g)
                flush(g)

        # tail fc on reclaimed psum banks, drains 3-way across engines
        state["tail"] = True
        with tc.tile_pool(name="fct", bufs=4, space="PSUM") as fct:
            def ps_tail():
                return fct.tile([128, 512], F32, name="ps_ft", tag="ps_ft")

            tail_gens = [gen_fc(c, psfn=ps_tail) for c in sorted(fcs)]
            fcs.clear()
            while tail_gens:
                g = tail_gens.pop(0)
                try:
                    next(g)
                    tail_gens.append(g)
                except StopIteration:
                    pass

    nc.compile()
    return nc


def _prep_in_maps(inputs):
    f32 = np.float32
    q = np.asarray(inputs["query"], f32)
    k = np.asarray(inputs["key"], f32)
    w_dq = np.asarray(inputs["w_dq"], f32)
    w_dkv = np.asarray(inputs["w_dkv"], f32)
    w_uq = np.asarray(inputs["w_uq"], f32)
    w_uk = np.asarray(inputs["w_uk"], f32)
    w_uv = np.asarray(inputs["w_uv"], f32)
    w_qr = np.asarray(inputs["w_qr"], f32)
    w_kr = np.asarray(inputs["w_kr"], f32)
    w_fc = np.asarray(inputs["w_fc"], f32)
    b_dq = np.asarray(inputs["b_dq"], f32)
    b_dkv = np.asarray(inputs["b_dkv"], f32)
    b_uq = np.asarray(inputs["b_uq"], f32)
    b_uk = np.asarray(inputs["b_uk"], f32)
    b_qr = np.asarray(inputs["b_qr"], f32)
    b_kr = np.asarray(inputs["b_kr"], f32)

    CT, ST = _rope_tables()
    st_signed = ST.copy()
    st_signed[0::2] *= -1.0

    with_bias = any(np.any(np.asarray(inputs[n])) for n in
                    ("b_dq", "b_dkv", "b_uq", "b_uk", "b_qr", "b_kr"))

    # fused projections
    A_q = w_dq @ w_uq        # [Z, UP]
    A_r = w_dq @ w_qr        # [Z, H*RHD]
    K_k = w_dkv @ w_uk       # [Z, UP]
    K_v = w_dkv @ w_uv       # [Z, UP]
    bq_c = b_dq @ w_uq + b_uq     # [UP]
    bq_r = b_dq @ w_qr + b_qr     # [H*RHD]
    bk_c = b_dkv @ w_uk + b_uk    # [UP]
    # k-rope bias is just b_kr (k_t_r = key@w_kr + b_kr); v bias handled in
    # the host epilogue.

    qTb = [q[b_].T.astype(bf16) for b_ in range(B)]
    kTb = [k[b_].T.astype(bf16) for b_ in range(B)]

    in_maps = []
    for core in range(NCORES):
        b_idx, grp = core // HPC, core % HPC
        h0 = HPC * grp
        Wq = np.zeros((Z, 512), f32)
        Wk = np.zeros((Z, 256), f32)
        Wv = np.zeros((Z, 256), f32)
        for i in range(HPC):
            hh = h0 + i
            Wq[:, 128 * i:128 * i + 64] = A_q[:, 64 * hh:64 * hh + 64]
            Wq[:, 128 * i + 64:128 * (i + 1)] = A_r[:, 64 * hh:64 * hh + 64]
            Wk[:, 64 * i:64 * (i + 1)] = K_k[:, 64 * hh:64 * hh + 64]
            Wv[:, 64 * i:64 * (i + 1)] = K_v[:, 64 * hh:64 * hh + 64]
        # wfc rows: r = 128*c + p -> head 2c + (p>=64), dim p%64
        Wfc = np.zeros((256, Z), f32)
        for c in range(2):
            for sub in range(2):
                hh = h0 + 2 * c + sub
                Wfc[128 * c + 64 * sub:128 * c + 64 * (sub + 1), :] = \
                    w_fc[64 * hh:64 * (hh + 1), :]
        m = {
            "qT": qTb[b_idx], "kT": kTb[b_idx],
            "wq": Wq.astype(bf16), "wk": Wk.astype(bf16),
            "wv": Wv.astype(bf16), "wkr": w_kr.astype(bf16),
            "ct1": np.concatenate([CT, CT], axis=0).astype(bf16),
            "st1": np.concatenate([st_signed, st_signed], axis=0).astype(bf16),
            "wfc": Wfc.astype(bf16),
            "ident": np.eye(128, dtype=f32).astype(bf16),
        }
        if with_bias:
            bq = np.zeros((128, HPC, S), f32)
            bk = np.zeros((128, HPC, S), f32)
            for i in range(HPC):
                hh = h0 + i
                bq[0:64, i, :] = bq_c[64 * hh:64 * hh + 64, None]
                bq[64:128, i, :] = (
                    bq_r[64 * hh:64 * hh + 64, None] * CT
                    + _partner_rows(bq_r[64 * hh:64 * hh + 64])[:, None] * ST)
                bk[0:64, i, :] = bk_c[64 * hh:64 * hh + 64, None]
                bk[64:128, i, :] = (b_kr[:, None] * CT
                                    + _partner_rows(b_kr)[:, None] * ST)
            m["biasq"] = bq.astype(bf16)
            m["biask4"] = bk.astype(bf16)
        in_maps.append(m)
    return in_maps, with_bias


def kernel(**inputs) -> np.ndarray:
    in_maps, with_bias = _prep_in_maps(inputs)

    key = ("nc", with_bias)
    if key not in _cache:
        _cache[key] = build_nc(with_bias)
    nc = _cache[key]

    res = run_bass_kernel_spmd(nc, in_maps, core_ids=list(range(NCORES)))

    f32 = np.float32
    out = np.zeros((B, S, Z), f32)
    for core in range(NCORES):
        out[core // HPC] += res.results[core]["outT"].T.astype(f32)

    w_fc = np.asarray(inputs["w_fc"], f32)
    b_v = (np.asarray(inputs["b_dkv"], f32) @ np.asarray(inputs["w_uv"], f32)
           + np.asarray(inputs["b_uv"], f32))
    bias = np.asarray(inputs["b_fc"], f32) + b_v @ w_fc
    out += bias[None, None, :]
    return out.astype(np.float32)
